# revision 2
# baseline (speedup 1.0000x reference)
"""Deformable transformer decoder layer for Trainium2 (8 NeuronCores).

Sharding: data-parallel over batch B=4 x token-half (2) -> 8 cores.
The layer's matmuls are grouped into 8 fused multi-job Bass launches
(baseline: 33 launches of one generic matmul):
  A: qk-proj + v-proj + circular-conv (9 taps PSUM-accumulated) + value-proj
  B: intra out-proj    C: mf-proj      D: inter qk-proj + v-proj
  E: inter out-proj    F: so/aw-proj   G: deform out-proj
  H: FFN l1 + bias + ReLU + l2 fused on-chip (hidden never leaves SBUF)
All matmuls run in bf16 with fp32 PSUM accumulation; softmax / layernorm /
bilinear-gather glue runs on host between launches.
"""

import os
import sys

import numpy as np

for _p in ("/opt/trn_rl_repo",):
    if _p not in sys.path:
        sys.path.insert(0, _p)

import ml_dtypes

import concourse.bass as bass
import concourse.mybir as mybir
from concourse.bass_utils import run_bass_kernel_spmd

BF16 = ml_dtypes.bfloat16

D = 256
H = 8
DH = D // H
L = 4
P = 4
NADJ = 4
DFF = 1024
SPATIAL_SHAPES = [(100, 134), (50, 67), (25, 34), (13, 17)]
LEVEL_START = [0, 13400, 16750, 17600]
LV = 17821
B, NQ, NP = 4, 100, 20
T = NQ * NP          # 2000 tokens per batch
M = T // 2           # 1000 tokens per core
MPAD = 1024
NCORES = 8
NSEQ = NQ // 2       # 50 intra sequences (len NP=20) per core
VROWS = 8960         # value-proj rows per core (70 x 128); 8*8960 >= 4*LV

_NCALLS = 0
_EXEC_NS = 0
_DEBUG = bool(os.environ.get("KDBG"))

f32 = mybir.dt.float32
bf16 = mybir.dt.bfloat16
COPY = mybir.ActivationFunctionType.Copy
RELU = mybir.ActivationFunctionType.Relu
IDENT = mybir.ActivationFunctionType.Identity


# =========================================================================
# Program builders.  Shared structure: sync issues input DMAs (dsem +16
# each, in declared order), PE runs jobs (each job = list of psum fills,
# round-robin over 8 banks with copy-done back-pressure), ACT copies each
# psum to an SBUF obuf (psem +1), sync DMAs each job's obuf out when its
# fills are done.
# =========================================================================

class _P:  # per-program trace
    def __init__(self, nc, ctx):
        self.nc = nc
        self.ctx = ctx
        self.in_dmas = []     # (sbuf_ap, dram_ap)
        self.jobs = []        # dicts
        self.n_in = 0

    def ld(self, name, rows, cols, dt):
        nc = self.nc
        dram = nc.declare_dram_parameter(name, [rows, cols], dt, isOutput=False)
        sb = self.ctx.enter_context(
            nc.sbuf_tensor(f"sb_{name}", [128, rows // 128, cols], dt))
        sem = self.ctx.enter_context(nc.semaphore(f"ds_{name}"))
        self.in_sems = getattr(self, "in_sems", {})
        self.in_sems[name] = sem
        self.in_dmas.append(
            (sb[:], dram[:].rearrange("(a p) m -> p a m", p=128), sem))
        self.n_in += 1
        return sb

    def add_job(self, name, mtiles, n, nsteps, lhsT_ap, rhs_ap,
                wait_in, out_dt=f32, func=None, bias=None, wait_fills=0,
                out_sb=None, dma_out=True):
        """Standard job: uniform mtiles on partition dim, shared rhs."""
        nc = self.nc
        nm = len(mtiles)
        if out_sb is None:
            out_sb = self.ctx.enter_context(
                nc.sbuf_tensor(f"ob_{name}", [128, nm, n], out_dt))
        fills = []
        for mb, msz in enumerate(mtiles):
            fills.append(dict(
                m=msz, n=n, nsteps=nsteps,
                lhsT=(lambda mb: lambda k: lhsT_ap(mb, k))(mb),
                rhs=(lambda mb: lambda k: rhs_ap(mb, k))(mb),
                out=out_sb[0:msz, mb, 0:n],
                bias=bias(mb) if bias is not None else None))
        out_dram = None
        if dma_out:
            out_dram = nc.declare_dram_parameter(
                f"o_{name}", [nm * 128, n], out_dt, isOutput=True)
        self.jobs.append(dict(
            name=name, fills=fills, wait_in=wait_in, func=func,
            wait_fills=wait_fills, out_sb=out_sb, out_dram=out_dram))
        return out_sb

    def add_raw_job(self, name, fills, wait_in, out_sb, out_rows, out_cols,
                    out_dt=f32, func=None, wait_fills=0):
        nc = self.nc
        out_dram = nc.declare_dram_parameter(
            f"o_{name}", [out_rows, out_cols], out_dt, isOutput=True)
        self.jobs.append(dict(
            name=name, fills=fills, wait_in=wait_in, func=func,
            wait_fills=wait_fills, out_sb=out_sb, out_dram=out_dram))

    def finish(self):
        nc = self.nc
        osem = self.ctx.enter_context(nc.semaphore("osem"))
        pes = self.ctx.enter_context(nc.semaphore("pes"))
        psem = self.ctx.enter_context(nc.semaphore("psem"))
        jobs = self.jobs
        with self.ctx, nc.Block() as block:

            @block.sync
            def _(sync):
                for sb_ap, dram_ap, sem in self.in_dmas:
                    sync.dma_start(out=sb_ap, in_=dram_ap).then_inc(sem, 16)
                fills_cum = 0
                n_out = 0
                for j in jobs:
                    fills_cum += len(j["fills"])
                    if j["out_dram"] is None:
                        continue
                    sync.wait_ge(psem, fills_cum)
                    sync.dma_start(
                        out=j["out_dram"][:].rearrange("(a p) m -> p a m", p=128),
                        in_=j["out_sb"][:],
                    ).then_inc(osem, 16)
                    n_out += 1
                sync.wait_ge(osem, 16 * n_out)
                for _, _, sem in self.in_dmas:
                    sync.wait_ge(sem, 16)

            @block.tensor
            def _(tensor):
                fill = 0
                for j in jobs:
                    for nm in j["wait_in"]:
                        tensor.wait_ge(self.in_sems[nm], 16)
                    if j["wait_fills"]:
                        tensor.wait_ge(psem, j["wait_fills"])
                    for fl in j["fills"]:
                        if fill >= 8:
                            tensor.wait_ge(psem, fill - 7)
                        ps = self.psums[fill % 8]
                        ns = fl["nsteps"]
                        for k in range(ns):
                            inst = tensor.matmul(
                                ps[0:fl["m"], 0:fl["n"]],
                                lhsT=fl["lhsT"](k),
                                rhs=fl["rhs"](k),
                                start=(k == 0),
                                stop=(k == ns - 1),
                            )
                        inst.then_inc(pes, 1)
                        fill += 1

            @block.scalar
            def _(scalar):
                fill = 0
                for j in jobs:
                    for fl in j["fills"]:
                        scalar.wait_ge(pes, fill + 1)
                        ps = self.psums[fill % 8]
                        func = j["func"] or COPY
                        kwargs = {}
                        if fl.get("bias") is not None:
                            kwargs["bias"] = fl["bias"]
                        scalar.activation(
                            fl["out"], ps[0:fl["m"], 0:fl["n"]], func, **kwargs,
                        ).then_inc(psem, 1)
                        fill += 1

        return nc


def _new_prog():
    import contextlib
    nc = bass.Bass()
    ctx = contextlib.ExitStack()
    p = _P(nc, ctx)
    p.psums = [ctx.enter_context(nc.psum_tensor(f"ps{i}", [128, 512], f32))
               for i in range(8)]
    return p


def _tok_job(p, name, x_sb, w_sb, n, wait_in):
    p.add_job(name, [128] * 8, n, 2,
              lambda mb, k: x_sb[:, k, mb * 128:(mb + 1) * 128],
              lambda mb, k: w_sb[:, k, 0:n],
              wait_in)


def build_A():
    p = _new_prog()
    x_qin = p.ld("x_qin", 256, MPAD, f32)
    x_x0 = p.ld("x_x0", 256, MPAD, f32)
    x_pad = p.ld("x_pad", 256, NSEQ * 28, f32)
    w_qk = p.ld("w_qk", 256, 512, f32)
    w_v = p.ld("w_v", 256, 256, f32)
    w_cc = p.ld("w_cc", 2304, 256, f32)

    _tok_job(p, "qk", x_qin, w_qk, 512, ["x_qin", "w_qk"])
    _tok_job(p, "v", x_x0, w_v, 256, ["x_x0", "w_v"])

    # conv, output-transposed: psum [128 out-ch, token cols]; stationary =
    # w_cc [in-ch, out-ch] tiles, moving = padded-token windows (2-axis AP).
    xp4 = x_pad[:].rearrange("p a (s c) -> p a s c", c=28)
    cc_sb = p.ctx.enter_context(
        p.nc.sbuf_tensor("ob_cc", [128, 2, MPAD], f32))
    fills = []
    col_tiles = [(0, 24), (24, 24), (48, 2)]     # (seq0, nseqs)
    for opt in range(2):
        for s0, ns in col_tiles:
            ncols = ns * 20
            fills.append(dict(
                m=128, n=ncols, nsteps=18,
                lhsT=(lambda opt: lambda k:
                      w_cc[:, k, opt * 128:(opt + 1) * 128])(opt),
                rhs=(lambda s0, ns: lambda k:
                     xp4[:, k % 2, s0:s0 + ns, (k // 2):(k // 2) + 20])(s0, ns),
                out=cc_sb[0:128, opt, s0 * 20:s0 * 20 + ncols]))
    p.add_raw_job("cc", fills, ["x_pad", "w_cc"], cc_sb, 256, MPAD)
    return p.finish()


def build_V():
    p = _new_prog()
    w_vp = p.ld("w_vp", 256, 256, f32)
    x_src = p.ld("x_src", 256, VROWS, f32)
    p.add_job("val", [128] * 70, 256, 2,
              lambda mb, k: x_src[:, k, mb * 128:(mb + 1) * 128],
              lambda mb, k: w_vp[:, k, 0:256], ["x_src", "w_vp"])
    return p.finish()


def build_simple(tag, nout):
    def b():
        p = _new_prog()
        x = p.ld("x", 256, MPAD, f32)
        w = p.ld("w", 256, nout, f32)
        _tok_job(p, tag, x, w, nout, ["x", "w"])
        return p.finish()
    return b


def build_D():
    p = _new_prog()
    x_qi = p.ld("x_qi", 256, MPAD, f32)
    x_y = p.ld("x_y", 256, MPAD, f32)
    w_qk = p.ld("w_qk", 256, 512, f32)
    w_v = p.ld("w_v", 256, 256, f32)
    _tok_job(p, "qk2", x_qi, w_qk, 512, ["x_qi", "w_qk"])
    _tok_job(p, "v2", x_y, w_v, 256, ["x_y", "w_v"])
    return p.finish()


def build_H():
    p = _new_prog()
    x_t = p.ld("x_t", 256, MPAD, f32)        # tgt2^T
    w_l1 = p.ld("w_l1", 256, DFF, f32)       # l1_w.T  (lhsT [in, dff])
    b_l1 = p.ld("b_l1", 128, 8, f32)         # l1_b reshaped [128, 8]
    w_l2 = p.ld("w_l2", DFF, 256, f32)       # l2_w.T  (rhs [dff, 256])

    # l1 transposed: fill f -> (dff tile dt=f//2, token chunk c=f%2)
    hT = p.add_job(
        "l1t", [128] * 16, 512, 2,
        lambda mb, k: w_l1[:, k, (mb // 2) * 128:(mb // 2) * 128 + 128],
        lambda mb, k: x_t[:, k, (mb % 2) * 512:(mb % 2) * 512 + 512],
        ["x_t", "w_l1", "b_l1"], func=RELU,
        bias=lambda mb: b_l1[:, 0, (mb // 2):(mb // 2) + 1],
        dma_out=False)

    # hT obuf layout [128, 16, 512]: (dt, c) at index dt*2+c; token col m of
    # dff row (dt*128+pp) lives at hT[pp, dt*2 + m//512, m%512].
    def l2_lhsT(mb, k):
        # need [128 dff rows of tile k, 128 tokens at mb*128..]
        c = (mb * 128) // 512
        off = (mb * 128) % 512
        return hT[:, k * 2 + c, off:off + 128]

    p.add_job("l2", [128] * 8, 256, 8, l2_lhsT,
              lambda mb, k: w_l2[:, k, 0:256],
              ["w_l2"], wait_fills=16)
    return p.finish()


_PROGS = {}


def _prog(key, builder):
    if key not in _PROGS:
        _PROGS[key] = builder()
    return _PROGS[key]


def _run(key, builder, in_maps, est_ns):
    global _NCALLS, _EXEC_NS
    nc = _prog(key, builder)
    res = run_bass_kernel_spmd(nc, in_maps, list(range(NCORES)))
    _NCALLS += 1
    _EXEC_NS += int(res.exec_time_ns) if res.exec_time_ns else est_ns
    return res.results


# =========================================================================
# Host-side helpers (numerics identical to the reference / baseline).
# =========================================================================

def _layer_norm(x, g, b, eps=1e-5):
    m = x.mean(-1, keepdims=True)
    v = ((x - m) ** 2).mean(-1, keepdims=True)
    return ((x - m) / np.sqrt(v + eps) * g + b).astype(np.float32)


def _softmax(x, axis=-1):
    m = x.max(axis=axis, keepdims=True)
    e = np.exp(x - m)
    return (e / e.sum(axis=axis, keepdims=True)).astype(np.float32)


def _attention(qp_, kp_, vp_):
    G, S, _ = qp_.shape
    sp = lambda t: t.reshape(G, S, H, DH).transpose(0, 2, 1, 3)
    q, k, v = sp(qp_), sp(kp_), sp(vp_)
    att = _softmax(np.einsum("ghqd,ghkd->ghqk", q, k) / np.sqrt(DH), -1)
    o = np.einsum("ghqk,ghkd->ghqd", att, v)
    return o.transpose(0, 2, 1, 3).reshape(G, S, D).astype(np.float32)


def _bilinear(vflat, Hl, Wl, x, y):
    x0 = np.floor(x)
    y0 = np.floor(y)
    lx = (x - x0).astype(np.float32)
    ly = (y - y0).astype(np.float32)
    x0 = x0.astype(np.int64)
    y0 = y0.astype(np.int64)
    out = 0.0
    for dy, wy in ((0, 1.0 - ly), (1, ly)):
        for dx, wx in ((0, 1.0 - lx), (1, lx)):
            xi = x0 + dx
            yi = y0 + dy
            valid = (xi >= 0) & (xi < Wl) & (yi >= 0) & (yi < Hl)
            idx = np.clip(yi, 0, Hl - 1) * Wl + np.clip(xi, 0, Wl - 1)
            gs = np.take_along_axis(vflat, idx[..., None], axis=1)
            out = out + gs * (wx * wy * valid)[..., None]
    return out.astype(np.float32)


def _xT(a):
    """[m<=1024, 256] f32 -> [256, 1024] f32 (transposed, zero-padded)."""
    out = np.zeros((256, MPAD), np.float32)
    out[:, :a.shape[0]] = a.T
    return out


def _tok_out(res_c, name, n):
    """Device [1024, n] -> [1000, n] f32."""
    return np.asarray(res_c[f"o_{name}"][:M]).astype(np.float32)


def _tok_launch(key, builder, X, Wt, nout, est_ns):
    """X [B, T, 256] @ Wt [256, nout<=512] via one shared 8-core launch.
    All single-matmul launches share one compiled [256x1024 @ 256x512]
    program; W is zero-padded to 512 columns."""
    wp = np.zeros((256, 512), np.float32)
    wp[:, :nout] = Wt
    in_maps = []
    for c in range(NCORES):
        b, g = divmod(c, 2)
        in_maps.append({"x": _xT(X[b, g * M:(g + 1) * M]), "w": wp})
    res = _run("tok512", build_simple("tok512", 512), in_maps, est_ns)
    out = np.empty((B, T, nout), np.float32)
    for c in range(NCORES):
        b, g = divmod(c, 2)
        out[b, g * M:(g + 1) * M] = _tok_out(res[c], "tok512", 512)[:, :nout]
    return out


# =========================================================================
# Main kernel.
# =========================================================================

def kernel(
    tgt, query_pos, query_pos_anchor, reference_points, src,
    src_spatial_shapes, level_start_index,
    ia_wi, ia_bi, ia_wo, ia_bo,
    cc_w, cc_b, bn_g, bn_b, bn_m, bn_v,
    ni_g, ni_b, mf_w, mf_b, nf_g, nf_b,
    in_wi, in_bi, in_wo, in_bo, nin_g, nin_b,
    so_w, so_b, aw_w, aw_b, vp_w, vp_b, op_w, op_b, nc_g, nc_b,
    l1_w, l1_b, l2_w, l2_b, n3_g, n3_b,
):
    f = lambda a: np.asarray(a, np.float32)
    tgt = f(tgt)
    qp = f(query_pos)
    qpa = f(query_pos_anchor)
    ref = f(reference_points)
    src = f(src)

    x0 = tgt.reshape(B, T, D)
    qpf = qp.reshape(B, T, D)
    qpaf = qpa.reshape(B, T, D)
    q_in = x0 + qpf

    # -------- launch A: qk, v, conv -- and launch V: value-proj --------
    srcpad = np.zeros((NCORES * VROWS, D), np.float32)
    srcpad[:B * LV] = src.reshape(B * LV, D)
    ccw_r = f(cc_w).transpose(2, 1, 0).reshape(2304, 256)  # [tap*256+kin, out]
    in_maps = []
    in_maps_v = []
    for c in range(NCORES):
        b, g = divmod(c, 2)
        sl = slice(g * M, (g + 1) * M)
        sc = q_in[b, sl].reshape(NSEQ, NP, D)
        xp = np.concatenate([sc[:, -NADJ:], sc, sc[:, :NADJ]], axis=1)
        in_maps.append({
            "x_qin": _xT(q_in[b, sl]),
            "x_x0": _xT(x0[b, sl]),
            "x_pad": np.ascontiguousarray(
                xp.transpose(2, 0, 1).reshape(256, NSEQ * 28)),
            "w_qk": f(ia_wi)[:2 * D].T.copy(),
            "w_v": f(ia_wi)[2 * D:].T.copy(),
            "w_cc": np.ascontiguousarray(ccw_r),
        })
        in_maps_v.append({
            "w_vp": f(vp_w).T.copy(),
            "x_src": srcpad[c * VROWS:(c + 1) * VROWS].T.copy(),
        })
    resV = _run("V", build_V, in_maps_v, 90_000)
    resA = _run("A", build_A, in_maps, 90_000)

    qk = np.empty((B, T, 512), np.float32)
    vproj = np.empty((B, T, 256), np.float32)
    conv = np.empty((B, T, 256), np.float32)
    valpad = np.empty((NCORES * VROWS, 256), np.float32)
    for c in range(NCORES):
        b, g = divmod(c, 2)
        sl = slice(g * M, (g + 1) * M)
        qk[b, sl] = _tok_out(resA[c], "qk", 512)
        vproj[b, sl] = _tok_out(resA[c], "v", 256)
        conv[b, sl] = np.asarray(resA[c]["o_cc"])[:, :M].T.astype(np.float32)
        valpad[c * VROWS:(c + 1) * VROWS] = \
            np.asarray(resV[c]["o_val"]).astype(np.float32)

    if _DEBUG:
        exp = q_in @ f(ia_wi)[:2 * D].T
        print("dbg qk err", np.abs(qk - exp).max() / np.abs(exp).std())
        expc = np.zeros((B, T, D), np.float32)
        xpf = np.concatenate(
            [q_in.reshape(B, NQ, NP, D)[:, :, -NADJ:],
             q_in.reshape(B, NQ, NP, D),
             q_in.reshape(B, NQ, NP, D)[:, :, :NADJ]], axis=2)
        for t in range(9):
            expc += xpf[:, :, t:t + NP].reshape(B, T, D) @ f(cc_w)[:, :, t].T
        print("dbg cc err", np.abs(conv - expc).max() / np.abs(expc).std())

    # ---------------- intra attention (host softmax) ----------------
    qprj = qk[..., :D] + f(ia_bi)[:D]
    kprj = qk[..., D:] + f(ia_bi)[D:2 * D]
    vprj = vproj + f(ia_bi)[2 * D:]
    o = _attention(
        qprj.reshape(B * NQ, NP, D),
        kprj.reshape(B * NQ, NP, D),
        vprj.reshape(B * NQ, NP, D),
    ).reshape(B, T, D)
    t_att = _tok_launch("t_att", build_simple("t_att", 256), o,
                        f(ia_wo).T, 256, 20_000) + f(ia_bo)

    # conv epilogue on host: bias + BN + ReLU
    convb = conv + f(cc_b)
    convb = (convb - f(bn_m)) / np.sqrt(f(bn_v) + 1e-5) * f(bn_g) + f(bn_b)
    t_cc = np.maximum(convb, 0.0)

    y = x0 + _layer_norm(t_att + t_cc, f(ni_g), f(ni_b))
    mf = _tok_launch("mf", build_simple("mf", 256), y, f(mf_w).T, 256,
                     20_000) + f(mf_b)
    y = y + _layer_norm(mf, f(nf_g), f(nf_b))

    # ---------------- inter attention ----------------
    q_in2 = y + qpaf
    in_maps = []
    for c in range(NCORES):
        b, g = divmod(c, 2)
        sl = slice(g * M, (g + 1) * M)
        in_maps.append({
            "x_qi": _xT(q_in2[b, sl]),
            "x_y": _xT(y[b, sl]),
            "w_qk": f(in_wi)[:2 * D].T.copy(),
            "w_v": f(in_wi)[2 * D:].T.copy(),
        })
    resD = _run("D", build_D, in_maps, 30_000)
    qk2 = np.empty((B, T, 512), np.float32)
    vproj2 = np.empty((B, T, 256), np.float32)
    for c in range(NCORES):
        b, g = divmod(c, 2)
        sl = slice(g * M, (g + 1) * M)
        qk2[b, sl] = _tok_out(resD[c], "qk2", 512)
        vproj2[b, sl] = _tok_out(resD[c], "v2", 256)

    qprj2 = (qk2[..., :D] + f(in_bi)[:D]).reshape(B, NQ, NP, D)
    kprj2 = (qk2[..., D:] + f(in_bi)[D:2 * D]).reshape(B, NQ, NP, D)
    vprj2 = (vproj2 + f(in_bi)[2 * D:]).reshape(B, NQ, NP, D)
    tonp = lambda a: a.transpose(0, 2, 1, 3).reshape(B * NP, NQ, D)
    o2 = _attention(tonp(qprj2), tonp(kprj2), tonp(vprj2))
    o2 = o2.reshape(B, NP, NQ, D).transpose(0, 2, 1, 3).reshape(B, T, D)
    t2 = _tok_launch("t2", build_simple("t2", 256), o2, f(in_wo).T, 256,
                     20_000) + f(in_bo)
    ti = _layer_norm(y + t2, f(nin_g), f(nin_b))

    # ---------------- deformable cross attention ----------------
    qc = ti + qpf
    proj = _tok_launch("proj", build_simple("proj", 384), qc,
                       np.concatenate([f(so_w), f(aw_w)], 0).T, 384, 20_000)
    offsets = (proj[..., :H * L * P * 2] + f(so_b)).reshape(B, T, H, L, P, 2)
    aw = _softmax(
        (proj[..., H * L * P * 2:] + f(aw_b)).reshape(B, T, H, L * P), -1
    ).reshape(B, T, H, L, P)
    value = (valpad[:B * LV] + f(vp_b)).reshape(B, LV, H, DH)

    refq = ref.reshape(B, T, L, 2)
    normalizer = np.array([[wl, hl] for hl, wl in SPATIAL_SHAPES], np.float32)
    loc = (refq[:, :, None, :, None, :]
           + offsets / normalizer[None, None, None, :, None, :])
    out_s = np.zeros((B, T, H, DH), np.float32)
    for lvl, (Hl, Wl) in enumerate(SPATIAL_SHAPES):
        s = LEVEL_START[lvl]
        vflat = (value[:, s:s + Hl * Wl]
                 .transpose(0, 2, 1, 3).reshape(B * H, Hl * Wl, DH))
        gxy = 2.0 * loc[:, :, :, lvl] - 1.0
        x = ((gxy[..., 0] + 1.0) / 2.0) * Wl - 0.5
        y_ = ((gxy[..., 1] + 1.0) / 2.0) * Hl - 0.5
        x = x.transpose(0, 2, 1, 3).reshape(B * H, T * P)
        y_ = y_.transpose(0, 2, 1, 3).reshape(B * H, T * P)
        samp = _bilinear(vflat, Hl, Wl, x, y_).reshape(B, H, T, P, DH)
        wgt = aw[:, :, :, lvl].transpose(0, 2, 1, 3)
        out_s += np.einsum("nhqp,nhqpd->nqhd", wgt, samp).astype(np.float32)
    sampled = out_s.reshape(B, T, D)
    t2d = _tok_launch("op", build_simple("op", 256), sampled, f(op_w).T,
                      256, 20_000) + f(op_b)
    tgt2 = _layer_norm(ti + t2d, f(nc_g), f(nc_b))

    # ---------------- FFN (fused l1+relu+l2 on device) ----------------
    in_maps = []
    for c in range(NCORES):
        b, g = divmod(c, 2)
        sl = slice(g * M, (g + 1) * M)
        in_maps.append({
            "x_t": _xT(tgt2[b, sl]),
            "w_l1": f(l1_w).T.copy(),
            "b_l1": np.ascontiguousarray(
                f(l1_b).reshape(8, 128).T).astype(np.float32),
            "w_l2": f(l2_w).T.copy(),
        })
    resH = _run("H", build_H, in_maps, 55_000)
    h2 = np.empty((B, T, 256), np.float32)
    for c in range(NCORES):
        b, g = divmod(c, 2)
        h2[b, g * M:(g + 1) * M] = _tok_out(resH[c], "l2", 256)
    if _DEBUG:
        hh = np.maximum(tgt2 @ f(l1_w).T + f(l1_b), 0.0)
        expf = hh @ f(l2_w).T
        print("dbg ffn err", np.abs(h2 - expf).max() / np.abs(expf).std())
    h2 = h2 + f(l2_b)
    out = _layer_norm(tgt2 + h2, f(n3_g), f(n3_b))
    return out.reshape(B, NQ, NP, D).astype(np.float32)


# revision 6
# speedup vs baseline: 1.1812x; 1.1812x over previous
"""Deformable transformer decoder layer for Trainium2 (8 NeuronCores).

Sharding: data-parallel over batch B=4 x token-half (2) -> 8 cores.
The layer's matmuls are grouped into 8 fused multi-job Bass launches
(baseline: 33 launches of one generic matmul):
  A: qk-proj + v-proj + circular-conv (9 taps PSUM-accumulated) + value-proj
  B: intra out-proj    C: mf-proj      D: inter qk-proj + v-proj
  E: inter out-proj    F: so/aw-proj   G: deform out-proj
  H: FFN l1 + bias + ReLU + l2 fused on-chip (hidden never leaves SBUF)
All matmuls run in bf16 with fp32 PSUM accumulation; softmax / layernorm /
bilinear-gather glue runs on host between launches.
"""

import os
import sys

import numpy as np

for _p in ("/opt/trn_rl_repo",):
    if _p not in sys.path:
        sys.path.insert(0, _p)

import ml_dtypes

import concourse.bass as bass
import concourse.mybir as mybir
from concourse.bass_utils import run_bass_kernel_spmd

BF16 = ml_dtypes.bfloat16

D = 256
H = 8
DH = D // H
L = 4
P = 4
NADJ = 4
DFF = 1024
SPATIAL_SHAPES = [(100, 134), (50, 67), (25, 34), (13, 17)]
LEVEL_START = [0, 13400, 16750, 17600]
LV = 17821
B, NQ, NP = 4, 100, 20
T = NQ * NP          # 2000 tokens per batch
M = T // 2           # 1000 tokens per core
MPAD = 1024
NCORES = 8
NSEQ = NQ // 2       # 50 intra sequences (len NP=20) per core
VROWS = 8960         # value-proj rows per core (70 x 128); 8*8960 >= 4*LV

_NCALLS = 0
_EXEC_NS = 0
_DEBUG = bool(os.environ.get("KDBG"))

f32 = mybir.dt.float32
bf16 = mybir.dt.bfloat16
COPY = mybir.ActivationFunctionType.Copy
RELU = mybir.ActivationFunctionType.Relu
IDENT = mybir.ActivationFunctionType.Identity


# =========================================================================
# Program builders.  Shared structure: sync issues input DMAs (dsem +16
# each, in declared order), PE runs jobs (each job = list of psum fills,
# round-robin over 8 banks with copy-done back-pressure), ACT copies each
# psum to an SBUF obuf (psem +1), sync DMAs each job's obuf out when its
# fills are done.
# =========================================================================

class _P:  # per-program trace
    def __init__(self, nc, ctx):
        self.nc = nc
        self.ctx = ctx
        self.in_dmas = []     # (sbuf_ap, dram_ap)
        self.jobs = []        # dicts
        self.n_in = 0

    def ld(self, name, rows, cols, dt):
        nc = self.nc
        dram = nc.declare_dram_parameter(name, [rows, cols], dt, isOutput=False)
        sb = self.ctx.enter_context(
            nc.sbuf_tensor(f"sb_{name}", [128, rows // 128, cols], dt))
        sem = self.ctx.enter_context(nc.semaphore(f"ds_{name}"))
        self.in_sems = getattr(self, "in_sems", {})
        self.in_sems[name] = sem
        self.in_dmas.append(
            (sb[:], dram[:].rearrange("(a p) m -> p a m", p=128), sem))
        self.n_in += 1
        return sb

    def add_job(self, name, mtiles, n, nsteps, lhsT_ap, rhs_ap,
                wait_in, out_dt=f32, func=None, bias=None, wait_fills=0,
                out_sb=None, dma_out=True):
        """Standard job: uniform mtiles on partition dim, shared rhs."""
        nc = self.nc
        nm = len(mtiles)
        if out_sb is None:
            out_sb = self.ctx.enter_context(
                nc.sbuf_tensor(f"ob_{name}", [128, nm, n], out_dt))
        fills = []
        for mb, msz in enumerate(mtiles):
            fills.append(dict(
                m=msz, n=n, nsteps=nsteps,
                lhsT=(lambda mb: lambda k: lhsT_ap(mb, k))(mb),
                rhs=(lambda mb: lambda k: rhs_ap(mb, k))(mb),
                out=out_sb[0:msz, mb, 0:n],
                bias=bias(mb) if bias is not None else None))
        out_dram = None
        if dma_out:
            out_dram = nc.declare_dram_parameter(
                f"o_{name}", [nm * 128, n], out_dt, isOutput=True)
        self.jobs.append(dict(
            name=name, fills=fills, wait_in=wait_in, func=func,
            wait_fills=wait_fills, out_sb=out_sb, out_dram=out_dram))
        return out_sb

    def add_raw_job(self, name, fills, wait_in, out_sb, out_rows, out_cols,
                    out_dt=f32, func=None, wait_fills=0):
        nc = self.nc
        out_dram = nc.declare_dram_parameter(
            f"o_{name}", [out_rows, out_cols], out_dt, isOutput=True)
        self.jobs.append(dict(
            name=name, fills=fills, wait_in=wait_in, func=func,
            wait_fills=wait_fills, out_sb=out_sb, out_dram=out_dram))

    def finish(self):
        nc = self.nc
        osem = self.ctx.enter_context(nc.semaphore("osem"))
        pes = self.ctx.enter_context(nc.semaphore("pes"))
        psem = self.ctx.enter_context(nc.semaphore("psem"))
        jobs = self.jobs
        with self.ctx, nc.Block() as block:

            @block.sync
            def _(sync):
                for sb_ap, dram_ap, sem in self.in_dmas:
                    sync.dma_start(out=sb_ap, in_=dram_ap).then_inc(sem, 16)
                fills_cum = 0
                n_out = 0
                for j in jobs:
                    fills_cum += len(j["fills"])
                    if j["out_dram"] is None:
                        continue
                    sync.wait_ge(psem, fills_cum)
                    sync.dma_start(
                        out=j["out_dram"][:].rearrange("(a p) m -> p a m", p=128),
                        in_=j["out_sb"][:],
                    ).then_inc(osem, 16)
                    n_out += 1
                sync.wait_ge(osem, 16 * n_out)
                for _, _, sem in self.in_dmas:
                    sync.wait_ge(sem, 16)

            @block.tensor
            def _(tensor):
                fill = 0
                for j in jobs:
                    for nm in j["wait_in"]:
                        tensor.wait_ge(self.in_sems[nm], 16)
                    if j["wait_fills"]:
                        tensor.wait_ge(psem, j["wait_fills"])
                    for fl in j["fills"]:
                        if fill >= 8:
                            tensor.wait_ge(psem, fill - 7)
                        ps = self.psums[fill % 8]
                        ns = fl["nsteps"]
                        for k in range(ns):
                            inst = tensor.matmul(
                                ps[0:fl["m"], 0:fl["n"]],
                                lhsT=fl["lhsT"](k),
                                rhs=fl["rhs"](k),
                                start=(k == 0),
                                stop=(k == ns - 1),
                            )
                        inst.then_inc(pes, 1)
                        fill += 1

            @block.scalar
            def _(scalar):
                fill = 0
                for j in jobs:
                    for fl in j["fills"]:
                        scalar.wait_ge(pes, fill + 1)
                        ps = self.psums[fill % 8]
                        func = j["func"] or COPY
                        kwargs = {}
                        if fl.get("bias") is not None:
                            kwargs["bias"] = fl["bias"]
                        scalar.activation(
                            fl["out"], ps[0:fl["m"], 0:fl["n"]], func, **kwargs,
                        ).then_inc(psem, 1)
                        fill += 1

        return nc


def _new_prog():
    import contextlib
    nc = bass.Bass()
    ctx = contextlib.ExitStack()
    p = _P(nc, ctx)
    p.psums = [ctx.enter_context(nc.psum_tensor(f"ps{i}", [128, 512], f32))
               for i in range(8)]
    return p


def _tok_job(p, name, x_sb, w_sb, n, wait_in):
    p.add_job(name, [128] * 8, n, 2,
              lambda mb, k: x_sb[:, k, mb * 128:(mb + 1) * 128],
              lambda mb, k: w_sb[:, k, 0:n],
              wait_in)


def build_A():
    p = _new_prog()
    x_qin = p.ld("x_qin", 256, MPAD, f32)
    x_x0 = p.ld("x_x0", 256, MPAD, f32)
    x_pad = p.ld("x_pad", 256, NSEQ * 28, f32)
    w_qk = p.ld("w_qk", 256, 512, f32)
    w_v = p.ld("w_v", 256, 256, f32)
    w_cc = p.ld("w_cc", 2304, 256, f32)

    _tok_job(p, "qk", x_qin, w_qk, 512, ["x_qin", "w_qk"])
    _tok_job(p, "v", x_x0, w_v, 256, ["x_x0", "w_v"])

    # conv, output-transposed: psum [128 out-ch, token cols]; stationary =
    # w_cc [in-ch, out-ch] tiles, moving = padded-token windows (2-axis AP).
    xp4 = x_pad[:].rearrange("p a (s c) -> p a s c", c=28)
    cc_sb = p.ctx.enter_context(
        p.nc.sbuf_tensor("ob_cc", [128, 2, MPAD], f32))
    fills = []
    col_tiles = [(0, 24), (24, 24), (48, 2)]     # (seq0, nseqs)
    for opt in range(2):
        for s0, ns in col_tiles:
            ncols = ns * 20
            fills.append(dict(
                m=128, n=ncols, nsteps=18,
                lhsT=(lambda opt: lambda k:
                      w_cc[:, k, opt * 128:(opt + 1) * 128])(opt),
                rhs=(lambda s0, ns: lambda k:
                     xp4[:, k % 2, s0:s0 + ns, (k // 2):(k // 2) + 20])(s0, ns),
                out=cc_sb[0:128, opt, s0 * 20:s0 * 20 + ncols]))
    p.add_raw_job("cc", fills, ["x_pad", "w_cc"], cc_sb, 256, MPAD)
    _val_job(p, 18)
    return p.finish()


def _val_job(p, nmt):
    """Piggyback nmt value-projection mtiles onto a launch."""
    xs = p.ld("x_srcv", 256, nmt * 128, f32)
    wv = p.ld("w_vp", 256, 256, f32)
    p.add_job("val", [128] * nmt, 256, 2,
              lambda mb, k: xs[:, k, mb * 128:(mb + 1) * 128],
              lambda mb, k: wv[:, k, 0:256], ["x_srcv", "w_vp"])


def build_simple(tag, nout, with_val=0):
    def b():
        p = _new_prog()
        x = p.ld("x", 256, MPAD, f32)
        w = p.ld("w", 256, nout, f32)
        _tok_job(p, tag, x, w, nout, ["x", "w"])
        if with_val:
            _val_job(p, with_val)
        return p.finish()
    return b


def build_D():
    p = _new_prog()
    x_qi = p.ld("x_qi", 256, MPAD, f32)
    x_y = p.ld("x_y", 256, MPAD, f32)
    w_qk = p.ld("w_qk", 256, 512, f32)
    w_v = p.ld("w_v", 256, 256, f32)
    _tok_job(p, "qk2", x_qi, w_qk, 512, ["x_qi", "w_qk"])
    _tok_job(p, "v2", x_y, w_v, 256, ["x_y", "w_v"])
    _val_job(p, 16)
    return p.finish()


def build_H():
    p = _new_prog()
    x_t = p.ld("x_t", 256, MPAD, f32)        # tgt2^T
    w_l1 = p.ld("w_l1", 256, DFF, f32)       # l1_w.T  (lhsT [in, dff])
    b_l1 = p.ld("b_l1", 128, 8, f32)         # l1_b reshaped [128, 8]
    w_l2 = p.ld("w_l2", DFF, 256, f32)       # l2_w.T  (rhs [dff, 256])

    # l1 transposed: fill f -> (dff tile dt=f//2, token chunk c=f%2)
    hT = p.add_job(
        "l1t", [128] * 16, 512, 2,
        lambda mb, k: w_l1[:, k, (mb // 2) * 128:(mb // 2) * 128 + 128],
        lambda mb, k: x_t[:, k, (mb % 2) * 512:(mb % 2) * 512 + 512],
        ["x_t", "w_l1", "b_l1"], func=RELU,
        bias=lambda mb: b_l1[:, 0, (mb // 2):(mb // 2) + 1],
        dma_out=False)

    # hT obuf layout [128, 16, 512]: (dt, c) at index dt*2+c; token col m of
    # dff row (dt*128+pp) lives at hT[pp, dt*2 + m//512, m%512].
    def l2_lhsT(mb, k):
        # need [128 dff rows of tile k, 128 tokens at mb*128..]
        c = (mb * 128) // 512
        off = (mb * 128) % 512
        return hT[:, k * 2 + c, off:off + 128]

    p.add_job("l2", [128] * 8, 256, 8, l2_lhsT,
              lambda mb, k: w_l2[:, k, 0:256],
              ["w_l2"], wait_fills=16)
    return p.finish()


_PROGS = {}


def _prog(key, builder):
    if key not in _PROGS:
        _PROGS[key] = builder()
    return _PROGS[key]


def _run(key, builder, in_maps, est_ns):
    global _NCALLS, _EXEC_NS
    nc = _prog(key, builder)
    res = run_bass_kernel_spmd(nc, in_maps, list(range(NCORES)))
    _NCALLS += 1
    _EXEC_NS += int(res.exec_time_ns) if res.exec_time_ns else est_ns
    return res.results


# =========================================================================
# Host-side helpers (numerics identical to the reference / baseline).
# =========================================================================

def _layer_norm(x, g, b, eps=1e-5):
    m = x.mean(-1, keepdims=True)
    v = ((x - m) ** 2).mean(-1, keepdims=True)
    return ((x - m) / np.sqrt(v + eps) * g + b).astype(np.float32)


def _softmax(x, axis=-1):
    m = x.max(axis=axis, keepdims=True)
    e = np.exp(x - m)
    return (e / e.sum(axis=axis, keepdims=True)).astype(np.float32)


def _attention(qp_, kp_, vp_):
    G, S, _ = qp_.shape
    sp = lambda t: t.reshape(G, S, H, DH).transpose(0, 2, 1, 3)
    q, k, v = sp(qp_), sp(kp_), sp(vp_)
    att = _softmax(np.einsum("ghqd,ghkd->ghqk", q, k) / np.sqrt(DH), -1)
    o = np.einsum("ghqk,ghkd->ghqd", att, v)
    return o.transpose(0, 2, 1, 3).reshape(G, S, D).astype(np.float32)


def _bilinear(vflat, Hl, Wl, x, y):
    x0 = np.floor(x)
    y0 = np.floor(y)
    lx = (x - x0).astype(np.float32)
    ly = (y - y0).astype(np.float32)
    x0 = x0.astype(np.int64)
    y0 = y0.astype(np.int64)
    out = 0.0
    for dy, wy in ((0, 1.0 - ly), (1, ly)):
        for dx, wx in ((0, 1.0 - lx), (1, lx)):
            xi = x0 + dx
            yi = y0 + dy
            valid = (xi >= 0) & (xi < Wl) & (yi >= 0) & (yi < Hl)
            idx = np.clip(yi, 0, Hl - 1) * Wl + np.clip(xi, 0, Wl - 1)
            gs = np.take_along_axis(vflat, idx[..., None], axis=1)
            out = out + gs * (wx * wy * valid)[..., None]
    return out.astype(np.float32)


def _xT(a):
    """[m<=1024, 256] f32 -> [256, 1024] f32 (transposed, zero-padded)."""
    out = np.zeros((256, MPAD), np.float32)
    out[:, :a.shape[0]] = a.T
    return out


def _tok_out(res_c, name, n):
    """Device [1024, n] -> [1000, n] f32."""
    return np.asarray(res_c[f"o_{name}"][:M]).astype(np.float32)


_VAL_SPANS = {"A": (0, 18), "t_att": (18, 36), "mf": (36, 54), "D": (54, 70)}
_SRCPAD = None
_VALPAD = None


def _val_inputs(launch, c):
    lo, hi = _VAL_SPANS[launch]
    sl = _SRCPAD[c * VROWS + lo * 128:c * VROWS + hi * 128]
    return np.ascontiguousarray(sl.T)


def _val_collect(launch, res):
    lo, hi = _VAL_SPANS[launch]
    for c in range(NCORES):
        _VALPAD[c * VROWS + lo * 128:c * VROWS + hi * 128] = \
            np.asarray(res[c]["o_val"]).astype(np.float32)


def _tok_launch(key, builder, X, Wt, nout, est_ns):
    """X [B, T, 256] @ Wt [256, nout<=512] via one 8-core launch.
    Single-matmul launches share compiled programs (256/256v/512-wide);
    W is zero-padded to the program width. Launches named in _VAL_SPANS
    also carry a slice of the value projection."""
    width = 256 if nout <= 256 else 512
    val = _VAL_SPANS.get(key)
    nmt = (val[1] - val[0]) if val else 0
    tag = f"tok{width}v{nmt}" if val else f"tok{width}"
    wp = np.zeros((256, width), np.float32)
    wp[:, :nout] = Wt
    in_maps = []
    for c in range(NCORES):
        b, g = divmod(c, 2)
        im = {"x": _xT(X[b, g * M:(g + 1) * M]), "w": wp}
        if val:
            im["x_srcv"] = _val_inputs(key, c)
            im["w_vp"] = _WVP
        in_maps.append(im)
    est = est_ns - (6_000 if width == 256 else 0) + (15_000 if val else 0)
    res = _run(tag, build_simple(tag, width, with_val=nmt), in_maps, est)
    if val:
        _val_collect(key, res)
    out = np.empty((B, T, nout), np.float32)
    for c in range(NCORES):
        b, g = divmod(c, 2)
        out[b, g * M:(g + 1) * M] = _tok_out(res[c], tag, width)[:, :nout]
    return out


# =========================================================================
# Main kernel.
# =========================================================================

def kernel(
    tgt, query_pos, query_pos_anchor, reference_points, src,
    src_spatial_shapes, level_start_index,
    ia_wi, ia_bi, ia_wo, ia_bo,
    cc_w, cc_b, bn_g, bn_b, bn_m, bn_v,
    ni_g, ni_b, mf_w, mf_b, nf_g, nf_b,
    in_wi, in_bi, in_wo, in_bo, nin_g, nin_b,
    so_w, so_b, aw_w, aw_b, vp_w, vp_b, op_w, op_b, nc_g, nc_b,
    l1_w, l1_b, l2_w, l2_b, n3_g, n3_b,
):
    f = lambda a: np.asarray(a, np.float32)
    tgt = f(tgt)
    qp = f(query_pos)
    qpa = f(query_pos_anchor)
    ref = f(reference_points)
    src = f(src)

    x0 = tgt.reshape(B, T, D)
    qpf = qp.reshape(B, T, D)
    qpaf = qpa.reshape(B, T, D)
    q_in = x0 + qpf

    # ---- launch A: qk, v, conv (+ value slice); value-proj rides on
    # launches A / t_att / mf / D (spans in _VAL_SPANS) ----
    global _SRCPAD, _VALPAD, _WVP
    _SRCPAD = np.zeros((NCORES * VROWS, D), np.float32)
    _SRCPAD[:B * LV] = src.reshape(B * LV, D)
    _VALPAD = np.empty((NCORES * VROWS, 256), np.float32)
    _WVP = f(vp_w).T.copy()
    ccw_r = f(cc_w).transpose(2, 1, 0).reshape(2304, 256)  # [tap*256+kin, out]
    in_maps = []
    for c in range(NCORES):
        b, g = divmod(c, 2)
        sl = slice(g * M, (g + 1) * M)
        sc = q_in[b, sl].reshape(NSEQ, NP, D)
        xp = np.concatenate([sc[:, -NADJ:], sc, sc[:, :NADJ]], axis=1)
        in_maps.append({
            "x_qin": _xT(q_in[b, sl]),
            "x_x0": _xT(x0[b, sl]),
            "x_pad": np.ascontiguousarray(
                xp.transpose(2, 0, 1).reshape(256, NSEQ * 28)),
            "w_qk": f(ia_wi)[:2 * D].T.copy(),
            "w_v": f(ia_wi)[2 * D:].T.copy(),
            "w_cc": np.ascontiguousarray(ccw_r),
            "x_srcv": _val_inputs("A", c),
            "w_vp": _WVP,
        })
    resA = _run("A", build_A, in_maps, 105_000)
    _val_collect("A", resA)

    qk = np.empty((B, T, 512), np.float32)
    vproj = np.empty((B, T, 256), np.float32)
    conv = np.empty((B, T, 256), np.float32)
    for c in range(NCORES):
        b, g = divmod(c, 2)
        sl = slice(g * M, (g + 1) * M)
        qk[b, sl] = _tok_out(resA[c], "qk", 512)
        vproj[b, sl] = _tok_out(resA[c], "v", 256)
        conv[b, sl] = np.asarray(resA[c]["o_cc"])[:, :M].T.astype(np.float32)

    if _DEBUG:
        exp = q_in @ f(ia_wi)[:2 * D].T
        print("dbg qk err", np.abs(qk - exp).max() / np.abs(exp).std())
        expc = np.zeros((B, T, D), np.float32)
        xpf = np.concatenate(
            [q_in.reshape(B, NQ, NP, D)[:, :, -NADJ:],
             q_in.reshape(B, NQ, NP, D),
             q_in.reshape(B, NQ, NP, D)[:, :, :NADJ]], axis=2)
        for t in range(9):
            expc += xpf[:, :, t:t + NP].reshape(B, T, D) @ f(cc_w)[:, :, t].T
        print("dbg cc err", np.abs(conv - expc).max() / np.abs(expc).std())

    # ---------------- intra attention (host softmax) ----------------
    qprj = qk[..., :D] + f(ia_bi)[:D]
    kprj = qk[..., D:] + f(ia_bi)[D:2 * D]
    vprj = vproj + f(ia_bi)[2 * D:]
    o = _attention(
        qprj.reshape(B * NQ, NP, D),
        kprj.reshape(B * NQ, NP, D),
        vprj.reshape(B * NQ, NP, D),
    ).reshape(B, T, D)
    t_att = _tok_launch("t_att", build_simple("t_att", 256), o,
                        f(ia_wo).T, 256, 20_000) + f(ia_bo)

    # conv epilogue on host: bias + BN + ReLU
    convb = conv + f(cc_b)
    convb = (convb - f(bn_m)) / np.sqrt(f(bn_v) + 1e-5) * f(bn_g) + f(bn_b)
    t_cc = np.maximum(convb, 0.0)

    y = x0 + _layer_norm(t_att + t_cc, f(ni_g), f(ni_b))
    mf = _tok_launch("mf", build_simple("mf", 256), y, f(mf_w).T, 256,
                     20_000) + f(mf_b)
    y = y + _layer_norm(mf, f(nf_g), f(nf_b))

    # ---------------- inter attention ----------------
    q_in2 = y + qpaf
    in_maps = []
    for c in range(NCORES):
        b, g = divmod(c, 2)
        sl = slice(g * M, (g + 1) * M)
        in_maps.append({
            "x_qi": _xT(q_in2[b, sl]),
            "x_y": _xT(y[b, sl]),
            "w_qk": f(in_wi)[:2 * D].T.copy(),
            "w_v": f(in_wi)[2 * D:].T.copy(),
            "x_srcv": _val_inputs("D", c),
            "w_vp": _WVP,
        })
    resD = _run("D", build_D, in_maps, 43_000)
    _val_collect("D", resD)
    qk2 = np.empty((B, T, 512), np.float32)
    vproj2 = np.empty((B, T, 256), np.float32)
    for c in range(NCORES):
        b, g = divmod(c, 2)
        sl = slice(g * M, (g + 1) * M)
        qk2[b, sl] = _tok_out(resD[c], "qk2", 512)
        vproj2[b, sl] = _tok_out(resD[c], "v2", 256)

    qprj2 = (qk2[..., :D] + f(in_bi)[:D]).reshape(B, NQ, NP, D)
    kprj2 = (qk2[..., D:] + f(in_bi)[D:2 * D]).reshape(B, NQ, NP, D)
    vprj2 = (vproj2 + f(in_bi)[2 * D:]).reshape(B, NQ, NP, D)
    tonp = lambda a: a.transpose(0, 2, 1, 3).reshape(B * NP, NQ, D)
    o2 = _attention(tonp(qprj2), tonp(kprj2), tonp(vprj2))
    o2 = o2.reshape(B, NP, NQ, D).transpose(0, 2, 1, 3).reshape(B, T, D)
    t2 = _tok_launch("t2", build_simple("t2", 256), o2, f(in_wo).T, 256,
                     20_000) + f(in_bo)
    ti = _layer_norm(y + t2, f(nin_g), f(nin_b))

    # ---------------- deformable cross attention ----------------
    qc = ti + qpf
    proj = _tok_launch("proj", build_simple("proj", 384), qc,
                       np.concatenate([f(so_w), f(aw_w)], 0).T, 384, 20_000)
    offsets = (proj[..., :H * L * P * 2] + f(so_b)).reshape(B, T, H, L, P, 2)
    aw = _softmax(
        (proj[..., H * L * P * 2:] + f(aw_b)).reshape(B, T, H, L * P), -1
    ).reshape(B, T, H, L, P)
    value = (_VALPAD[:B * LV] + f(vp_b)).reshape(B, LV, H, DH)

    refq = ref.reshape(B, T, L, 2)
    normalizer = np.array([[wl, hl] for hl, wl in SPATIAL_SHAPES], np.float32)
    loc = (refq[:, :, None, :, None, :]
           + offsets / normalizer[None, None, None, :, None, :])
    out_s = np.zeros((B, T, H, DH), np.float32)
    for lvl, (Hl, Wl) in enumerate(SPATIAL_SHAPES):
        s = LEVEL_START[lvl]
        vflat = (value[:, s:s + Hl * Wl]
                 .transpose(0, 2, 1, 3).reshape(B * H, Hl * Wl, DH))
        gxy = 2.0 * loc[:, :, :, lvl] - 1.0
        x = ((gxy[..., 0] + 1.0) / 2.0) * Wl - 0.5
        y_ = ((gxy[..., 1] + 1.0) / 2.0) * Hl - 0.5
        x = x.transpose(0, 2, 1, 3).reshape(B * H, T * P)
        y_ = y_.transpose(0, 2, 1, 3).reshape(B * H, T * P)
        samp = _bilinear(vflat, Hl, Wl, x, y_).reshape(B, H, T, P, DH)
        wgt = aw[:, :, :, lvl].transpose(0, 2, 1, 3)
        out_s += np.einsum("nhqp,nhqpd->nqhd", wgt, samp).astype(np.float32)
    sampled = out_s.reshape(B, T, D)
    t2d = _tok_launch("op", build_simple("op", 256), sampled, f(op_w).T,
                      256, 20_000) + f(op_b)
    tgt2 = _layer_norm(ti + t2d, f(nc_g), f(nc_b))

    # ---------------- FFN (fused l1+relu+l2 on device) ----------------
    in_maps = []
    for c in range(NCORES):
        b, g = divmod(c, 2)
        sl = slice(g * M, (g + 1) * M)
        in_maps.append({
            "x_t": _xT(tgt2[b, sl]),
            "w_l1": f(l1_w).T.copy(),
            "b_l1": np.ascontiguousarray(
                f(l1_b).reshape(8, 128).T).astype(np.float32),
            "w_l2": f(l2_w).T.copy(),
        })
    resH = _run("H", build_H, in_maps, 55_000)
    h2 = np.empty((B, T, 256), np.float32)
    for c in range(NCORES):
        b, g = divmod(c, 2)
        h2[b, g * M:(g + 1) * M] = _tok_out(resH[c], "l2", 256)
    if _DEBUG:
        hh = np.maximum(tgt2 @ f(l1_w).T + f(l1_b), 0.0)
        expf = hh @ f(l2_w).T
        print("dbg ffn err", np.abs(h2 - expf).max() / np.abs(expf).std())
    h2 = h2 + f(l2_b)
    out = _layer_norm(tgt2 + h2, f(n3_g), f(n3_b))
    return out.reshape(B, NQ, NP, D).astype(np.float32)
